# revision 4
# baseline (speedup 1.0000x reference)
"""Trainium2 Bass kernel for AdvancedMoEMixtureLoRA.

Reference computation (per token t of N = 4*2048 = 8192, D = 4096):
    z        = x @ A_w.T                       [N, 16]
    M        = 8 * (x @ M_w.T + M_b)           [N, 256] -> [N, 16, 16]
    z_mixed  = M @ z  (per token matvec)       [N, 16]
    out      = 128 * z_mixed @ B_w.T           [N, 4096]

Strategy: pure data parallel over tokens (1024 tokens per core, weights
replicated, no collectives).  Host-side prep (free, not on HW critical
path): transpose x to d-major, cast everything to bf16, fuse A_w/M_w
into one [4096, 272] weight, fold all scalar factors into the weights.

Per-core kernel, per 128-token chunk:
  - 32 accumulating matmuls (stationary = xT d-tile, moving = fused W)
    + one K=1 matmul adding the M_b bias row -> PSUM [128, 272]
    (cols 0:256 = M, cols 256:272 = z)
  - DVE mixing: P = M * broadcast(z), grouped reduce over j -> z_mixed
  - PE transpose z_mixed -> [16, 128], then 8 matmuls against
    B_w.T [16, 4096] -> out rows [128, 4096] in PSUM
  - DVE/ACT evacuate PSUM -> SBUF bf16, DMA store contiguous rows
"""

import sys

if "/opt/trn_rl_repo" not in sys.path:
    sys.path.insert(0, "/opt/trn_rl_repo")

import ml_dtypes
import numpy as np

import concourse.bass as bass
import concourse.tile as tile
from concourse import bacc, mybir
from concourse.bass_utils import run_bass_kernel_spmd

N_CORES = 8
B, S, D = 4, 2048, 4096
N_TOK = B * S                # 8192
TPC = N_TOK // N_CORES       # tokens per core = 1024
CHUNK = 128                  # tokens per PSUM chunk
NCHUNK = TPC // CHUNK        # 8
RH = 16                      # lora rank*heads
MDIM = RH * RH               # 256
WCOLS = MDIM + RH            # 272 fused output cols (M | z)
KD = D // 128                # 32 d-chunks
OUT_D = 4096

BF = mybir.dt.bfloat16
F32 = mybir.dt.float32
NPBF = ml_dtypes.bfloat16


def build_nc():
    nc = bacc.Bacc("TRN2", target_bir_lowering=False, debug=False)
    # pre-swizzled on host: xsw[p, k*TPC + t] = xT[k*128 + p, t]
    xsw = nc.dram_tensor("xsw", [128, KD * TPC], BF, kind="ExternalInput").ap()
    # pre-swizzled: wsw[p, k*WCOLS + m] = W.T[k*128 + p, m]
    wsw = nc.dram_tensor("wsw", [128, KD * WCOLS], BF, kind="ExternalInput").ap()
    mbr = nc.dram_tensor("mbr", [1, WCOLS], BF, kind="ExternalInput").ap()
    bT = nc.dram_tensor("bT", [RH, OUT_D], BF, kind="ExternalInput").ap()
    ones = nc.dram_tensor("ones", [1, CHUNK], BF, kind="ExternalInput").ap()
    ident = nc.dram_tensor("ident", [CHUNK, CHUNK], F32, kind="ExternalInput").ap()
    out = nc.dram_tensor("out", [TPC, OUT_D], BF, kind="ExternalOutput").ap()

    with tile.TileContext(nc) as tc:
        with (
            tc.tile_pool(name="xpool", bufs=1) as xpool,
            tc.tile_pool(name="wpool", bufs=1) as wpool,
            tc.tile_pool(name="cpool", bufs=1) as cpool,
            tc.tile_pool(name="mix", bufs=2) as mixpool,
            tc.tile_pool(name="osb", bufs=3) as opool,
            tc.tile_pool(name="am", bufs=4, space="PSUM") as ampool,
            tc.tile_pool(name="tp", bufs=1, space="PSUM") as tpool,
            tc.tile_pool(name="bp", bufs=3, space="PSUM") as bpool,
        ):
            # x loads on the SP HWDGE queue (critical path for PE start)
            xsb = xpool.tile([128, KD, TPC], BF)
            xflat = xsb[:].rearrange("p k t -> p (k t)")
            GCOL = 4 * TPC  # 4 d-tiles per 1 MiB load
            for g in range(8):
                nc.sync.dma_start(
                    xflat[:, g * GCOL:(g + 1) * GCOL], xsw[:, g * GCOL:(g + 1) * GCOL]
                )

            # weights / constants on the SWDGE (gpsimd) queue — runs
            # concurrently with the x loads
            wsb = wpool.tile([128, KD, WCOLS], BF)
            nc.gpsimd.dma_start(wsb[:].rearrange("p k m -> p (k m)"), wsw)
            mbsb = cpool.tile([1, WCOLS], BF)
            nc.gpsimd.dma_start(mbsb[:], mbr)
            btsb = cpool.tile([RH, OUT_D], BF)
            nc.gpsimd.dma_start(btsb[:], bT)
            onesb = cpool.tile([1, CHUNK], BF)
            nc.gpsimd.dma_start(onesb[:], ones)
            idsb = cpool.tile([CHUNK, CHUNK], F32)
            nc.gpsimd.dma_start(idsb[:], ident)

            for c in range(NCHUNK):
                tok = slice(c * CHUNK, (c + 1) * CHUNK)
                # fused A/M matmul: out [128 tok, 272], contract over d
                am = ampool.tile([128, WCOLS], F32)
                for k in range(KD):
                    nc.tensor.matmul(
                        am[:], lhsT=xsb[:, k, tok], rhs=wsb[:, k, :],
                        start=(k == 0), stop=False,
                    )
                # bias row via K=1 matmul (ones.T @ mb_row)
                nc.tensor.matmul(am[:], lhsT=onesb[:], rhs=mbsb[:], start=False, stop=True)

                # z -> SBUF (scalar engine, tiny)
                z_sb = mixpool.tile([128, RH], F32, tag="z")
                nc.scalar.copy(z_sb[:], am[:, MDIM:WCOLS])

                # P[p, i, j] = M[p, i, j] * z[p, j]
                p_sb = mixpool.tile([128, MDIM], BF, tag="p")
                nc.vector.tensor_mul(
                    p_sb[:].rearrange("p (i j) -> p i j", i=RH),
                    am[:, 0:MDIM].rearrange("p (i j) -> p i j", i=RH),
                    z_sb[:].unsqueeze(1).broadcast_to([128, RH, RH]),
                )
                # z_mixed[p, i] = sum_j P[p, i, j]
                zm = mixpool.tile([128, RH], F32, tag="zm")
                nc.vector.tensor_reduce(
                    zm[:], p_sb[:].rearrange("p (i j) -> p i j", i=RH),
                    axis=mybir.AxisListType.X, op=mybir.AluOpType.add,
                )

                # transpose z_mixed -> [16, 128] for the B matmul stationary
                zt_ps = tpool.tile([RH, CHUNK], F32)
                nc.tensor.transpose(zt_ps[:], zm[:], idsb[:])
                zt_sb = mixpool.tile([RH, CHUNK], BF, tag="zt")
                nc.scalar.copy(zt_sb[:], zt_ps[:])

                # out rows = z_mixed @ B_w.T : 8 matmuls of [16,128].T @ [16,512]
                o_sb = opool.tile([128, OUT_D], BF)
                for ob in range(8):
                    osl = slice(ob * 512, (ob + 1) * 512)
                    bp = bpool.tile([128, 512], F32)
                    nc.tensor.matmul(bp[:], lhsT=zt_sb[:], rhs=btsb[:, osl], start=True, stop=True)
                    # split PSUM evacuation between DVE and ACT
                    if ob % 2 == 0:
                        nc.vector.tensor_copy(o_sb[:, osl], bp[:])
                    else:
                        nc.scalar.copy(o_sb[:, osl], bp[:])
                # stores on the ACT HWDGE queue — independent FIFO from x loads
                nc.scalar.dma_start(out[tok, :], o_sb[:])

    nc.compile()
    return nc


_NC = None


def _get_nc():
    global _NC
    if _NC is None:
        _NC = build_nc()
    return _NC


def make_in_maps(x, A_w, B_w, M_w, M_b):
    x = np.asarray(x, dtype=np.float32)
    A_w = np.asarray(A_w, dtype=np.float32)
    B_w = np.asarray(B_w, dtype=np.float32)
    M_w = np.asarray(M_w, dtype=np.float32)
    M_b = np.asarray(M_b, dtype=np.float32)

    # fold scales: M' = x @ (8 M_w).T + 8 M_b ; out = z_mixed @ (128 B_w).T
    W = np.concatenate([8.0 * M_w, A_w], axis=0)              # [272, 4096]
    wT_np = W.T.astype(NPBF)                                  # [4096, 272]
    # swizzle to [128, k*272 + m] so each SBUF partition line is contiguous
    wsw_np = np.ascontiguousarray(
        wT_np.reshape(KD, 128, WCOLS).transpose(1, 0, 2).reshape(128, KD * WCOLS)
    )
    mb_np = np.concatenate([8.0 * M_b, np.zeros(RH, np.float32)]).reshape(1, WCOLS).astype(NPBF)
    bT_np = np.ascontiguousarray((128.0 * B_w).T).astype(NPBF)  # [16, 4096]
    ones_np = np.ones((1, CHUNK), dtype=NPBF)
    id_np = np.eye(CHUNK, dtype=np.float32)

    xf = x.reshape(N_TOK, D)
    in_maps = []
    for c in range(N_CORES):
        shard = xf[c * TPC:(c + 1) * TPC]                     # [1024, 4096]
        xT_np = shard.T.astype(NPBF)                          # [4096, 1024]
        xsw_np = np.ascontiguousarray(
            xT_np.reshape(KD, 128, TPC).transpose(1, 0, 2).reshape(128, KD * TPC)
        )
        in_maps.append({
            "xsw": xsw_np, "wsw": wsw_np, "mbr": mb_np, "bT": bT_np,
            "ones": ones_np, "ident": id_np,
        })
    return in_maps


def assemble_out(results):
    outs = [np.asarray(results[i]["out"], dtype=np.float32) for i in range(N_CORES)]
    return np.concatenate(outs, axis=0).reshape(B, S, OUT_D)


def kernel(x, A_w, B_w, M_w, M_b):
    nc = _get_nc()
    in_maps = make_in_maps(x, A_w, B_w, M_w, M_b)
    res = run_bass_kernel_spmd(nc, in_maps, core_ids=list(range(N_CORES)))
    return assemble_out(res.results)


# revision 5
# speedup vs baseline: 1.2904x; 1.2904x over previous
"""Trainium2 Bass kernel for AdvancedMoEMixtureLoRA.

Reference computation (per token t of N = 4*2048 = 8192, D = 4096):
    z        = x @ A_w.T                       [N, 16]
    M        = 8 * (x @ M_w.T + M_b)           [N, 256] -> [N, 16, 16]
    z_mixed  = M @ z  (per token matvec)       [N, 16]
    out      = 128 * z_mixed @ B_w.T           [N, 4096]

Strategy: pure data parallel over tokens (1024 tokens per core, weights
replicated, no collectives).  Host-side prep (free, not on HW critical
path): transpose x to d-major per 128-token slab, cast everything to
bf16, fuse A_w/M_w into one [4096, 272] weight, fold all scalar factors
into the weights.

Per-core kernel, per 128-token chunk (8 chunks):
  - load x slab [4096, 128] (1 MiB, contiguous per-partition lines)
  - 1 bias matmul (K=1 ones row x M_b row) + 32 accumulating matmuls
    (stationary = x d-tile, moving = fused W [128, 272]) -> PSUM
    [128 tok, 272] = (M | z)
  - DVE mixing: P = M * broadcast(z), grouped reduce over j -> z_mixed
  - PE transpose z_mixed -> [16, 128], then 8 matmuls against
    B_w.T [16, 4096] -> out rows [128 tok, 4096] in PSUM
  - DVE/ACT evacuate PSUM -> SBUF bf16 (1024-wide ops), DMA store rows
"""

import sys

if "/opt/trn_rl_repo" not in sys.path:
    sys.path.insert(0, "/opt/trn_rl_repo")

import ml_dtypes
import numpy as np

import concourse.bass as bass
import concourse.tile as tile
from concourse import bacc, mybir
from concourse.bass_utils import run_bass_kernel_spmd

N_CORES = 8
B, S, D = 4, 2048, 4096
N_TOK = B * S                # 8192
TPC = N_TOK // N_CORES       # tokens per core = 1024
CHUNK = 128                  # tokens per PSUM chunk
NCHUNK = TPC // CHUNK        # 8
RH = 16                      # lora rank*heads
MDIM = RH * RH               # 256
WCOLS = MDIM + RH            # 272 fused output cols (M | z)
KD = D // 128                # 32 d-chunks
OUT_D = 4096

BF = mybir.dt.bfloat16
F32 = mybir.dt.float32
NPBF = ml_dtypes.bfloat16


def build_nc():
    nc = bacc.Bacc("TRN2", target_bir_lowering=False, debug=False)
    # host-swizzled x: xsw[p, c*(KD*CHUNK) + k*CHUNK + t] = x[c*CHUNK + t, k*128 + p]
    xsw = nc.dram_tensor("xsw", [128, NCHUNK * KD * CHUNK], BF, kind="ExternalInput").ap()
    # host-swizzled W: wsw[p, k*WCOLS + m] = W.T[k*128 + p, m]
    wsw = nc.dram_tensor("wsw", [128, KD * WCOLS], BF, kind="ExternalInput").ap()
    mbr = nc.dram_tensor("mbr", [1, WCOLS], BF, kind="ExternalInput").ap()
    bT = nc.dram_tensor("bT", [RH, OUT_D], BF, kind="ExternalInput").ap()
    ones = nc.dram_tensor("ones", [1, CHUNK], BF, kind="ExternalInput").ap()
    ident = nc.dram_tensor("ident", [CHUNK, CHUNK], F32, kind="ExternalInput").ap()
    out = nc.dram_tensor("out", [TPC, OUT_D], BF, kind="ExternalOutput").ap()

    SLAB = KD * CHUNK  # 4096 cols per token-slab

    with tile.TileContext(nc) as tc:
        with (
            tc.tile_pool(name="xpool", bufs=3) as xpool,
            tc.tile_pool(name="wpool", bufs=1) as wpool,
            tc.tile_pool(name="cpool", bufs=1) as cpool,
            tc.tile_pool(name="mix", bufs=2) as mixpool,
            tc.tile_pool(name="osb", bufs=3) as opool,
            tc.tile_pool(name="am", bufs=3, space="PSUM") as ampool,
            tc.tile_pool(name="tp", bufs=1, space="PSUM") as tpool,
            tc.tile_pool(name="bp", bufs=2, space="PSUM") as bpool,
        ):
            # weights / constants on the SWDGE (gpsimd) queue — concurrent
            # with the x slab loads on the SP HWDGE queue
            wsb = wpool.tile([128, KD, WCOLS], BF)
            wflat = wsb[:].rearrange("p k m -> p (k m)")
            WQ = 8 * WCOLS
            for q in range(4):
                nc.gpsimd.dma_start(wflat[:, q * WQ:(q + 1) * WQ], wsw[:, q * WQ:(q + 1) * WQ])
            mbsb = cpool.tile([1, WCOLS], BF)
            nc.gpsimd.dma_start(mbsb[:], mbr)
            btsb = cpool.tile([RH, OUT_D], BF)
            nc.gpsimd.dma_start(btsb[:], bT)
            onesb = cpool.tile([1, CHUNK], BF)
            nc.gpsimd.dma_start(onesb[:], ones)
            idsb = cpool.tile([CHUNK, CHUNK], F32)
            nc.gpsimd.dma_start(idsb[:], ident)

            for c in range(NCHUNK):
                tok = slice(c * CHUNK, (c + 1) * CHUNK)
                # x slab: all of d for tokens [c*128, (c+1)*128)
                xs = xpool.tile([128, KD, CHUNK], BF)
                nc.sync.dma_start(
                    xs[:].rearrange("p k t -> p (k t)"),
                    xsw[:, c * SLAB:(c + 1) * SLAB],
                )

                # fused A/M matmul: out [128 tok, 272], contract over d.
                # bias row first (K=1, only needs mbr+ones), then 32 d-tiles.
                am = ampool.tile([128, WCOLS], F32)
                nc.tensor.matmul(am[:], lhsT=onesb[:], rhs=mbsb[:], start=True, stop=False)
                for k in range(KD):
                    nc.tensor.matmul(
                        am[:], lhsT=xs[:, k, :], rhs=wsb[:, k, :],
                        start=False, stop=(k == KD - 1),
                    )

                # z -> SBUF (scalar engine, tiny)
                z_sb = mixpool.tile([128, RH], F32, tag="z")
                nc.scalar.copy(z_sb[:], am[:, MDIM:WCOLS])

                # P[p, i, j] = M[p, i, j] * z[p, j]
                p_sb = mixpool.tile([128, MDIM], BF, tag="p")
                nc.vector.tensor_mul(
                    p_sb[:].rearrange("p (i j) -> p i j", i=RH),
                    am[:, 0:MDIM].rearrange("p (i j) -> p i j", i=RH),
                    z_sb[:].unsqueeze(1).broadcast_to([128, RH, RH]),
                )
                # z_mixed[p, i] = sum_j P[p, i, j]
                zm = mixpool.tile([128, RH], F32, tag="zm")
                nc.vector.tensor_reduce(
                    zm[:], p_sb[:].rearrange("p (i j) -> p i j", i=RH),
                    axis=mybir.AxisListType.X, op=mybir.AluOpType.add,
                )

                # transpose z_mixed -> [16, 128] for the B matmul stationary
                zt_ps = tpool.tile([RH, CHUNK], F32)
                nc.tensor.transpose(zt_ps[:], zm[:], idsb[:])
                zt_sb = mixpool.tile([RH, CHUNK], BF, tag="zt")
                nc.scalar.copy(zt_sb[:], zt_ps[:])

                # out rows = z_mixed @ B_w.T : 8 matmuls of [16,128].T @ [16,512]
                o_sb = opool.tile([128, OUT_D], BF)
                for h in range(4):
                    bp = bpool.tile([128, 1024], F32)  # 2 PSUM banks
                    for j in range(2):
                        ob = 2 * h + j
                        nc.tensor.matmul(
                            bp[:, j * 512:(j + 1) * 512],
                            lhsT=zt_sb[:], rhs=btsb[:, ob * 512:(ob + 1) * 512],
                            start=True, stop=True,
                        )
                    osl = slice(h * 1024, (h + 1) * 1024)
                    # split PSUM evacuation between DVE and ACT
                    if h % 2 == 0:
                        nc.vector.tensor_copy(o_sb[:, osl], bp[:])
                    else:
                        nc.scalar.copy(o_sb[:, osl], bp[:])
                # stores on the ACT HWDGE queue — independent FIFO from x loads
                nc.scalar.dma_start(out[tok, :], o_sb[:])

    nc.compile()
    return nc


_NC = None


def _get_nc():
    global _NC
    if _NC is None:
        _NC = build_nc()
    return _NC


def make_in_maps(x, A_w, B_w, M_w, M_b):
    x = np.asarray(x, dtype=np.float32)
    A_w = np.asarray(A_w, dtype=np.float32)
    B_w = np.asarray(B_w, dtype=np.float32)
    M_w = np.asarray(M_w, dtype=np.float32)
    M_b = np.asarray(M_b, dtype=np.float32)

    # fold scales: M' = x @ (8 M_w).T + 8 M_b ; out = z_mixed @ (128 B_w).T
    W = np.concatenate([8.0 * M_w, A_w], axis=0)              # [272, 4096]
    wT_np = W.T.astype(NPBF)                                  # [4096, 272]
    # swizzle to [128, k*272 + m] so each SBUF partition line is contiguous
    wsw_np = np.ascontiguousarray(
        wT_np.reshape(KD, 128, WCOLS).transpose(1, 0, 2).reshape(128, KD * WCOLS)
    )
    mb_np = np.concatenate([8.0 * M_b, np.zeros(RH, np.float32)]).reshape(1, WCOLS).astype(NPBF)
    bT_np = np.ascontiguousarray((128.0 * B_w).T).astype(NPBF)  # [16, 4096]
    ones_np = np.ones((1, CHUNK), dtype=NPBF)
    id_np = np.eye(CHUNK, dtype=np.float32)

    xf = x.reshape(N_TOK, D)
    in_maps = []
    for core in range(N_CORES):
        shard = xf[core * TPC:(core + 1) * TPC].astype(NPBF)  # [1024, 4096]
        # xsw[p, c*4096 + k*128 + t] = shard[c*128 + t, k*128 + p]
        xsw_np = np.ascontiguousarray(
            shard.reshape(NCHUNK, CHUNK, KD, 128)             # [c, t, k, p]
            .transpose(3, 0, 2, 1)                            # [p, c, k, t]
            .reshape(128, NCHUNK * KD * CHUNK)
        )
        in_maps.append({
            "xsw": xsw_np, "wsw": wsw_np, "mbr": mb_np, "bT": bT_np,
            "ones": ones_np, "ident": id_np,
        })
    return in_maps


def assemble_out(results):
    outs = [np.asarray(results[i]["out"], dtype=np.float32) for i in range(N_CORES)]
    return np.concatenate(outs, axis=0).reshape(B, S, OUT_D)


def kernel(x, A_w, B_w, M_w, M_b):
    nc = _get_nc()
    in_maps = make_in_maps(x, A_w, B_w, M_w, M_b)
    res = run_bass_kernel_spmd(nc, in_maps, core_ids=list(range(N_CORES)))
    return assemble_out(res.results)


# revision 6
# speedup vs baseline: 1.3947x; 1.0808x over previous
"""Trainium2 Bass kernel for AdvancedMoEMixtureLoRA.

Reference computation (per token t of N = 4*2048 = 8192, D = 4096):
    z        = x @ A_w.T                       [N, 16]
    M        = 8 * (x @ M_w.T + M_b)           [N, 256] -> [N, 16, 16]
    z_mixed  = M @ z  (per token matvec)       [N, 16]
    out      = 128 * z_mixed @ B_w.T           [N, 4096]

Strategy: pure data parallel over tokens (1024 tokens per core, weights
replicated, no collectives).  Host-side prep (free, not on HW critical
path): transpose x to d-major per 128-token slab, cast everything to
bf16, fuse A_w/M_w into one [4096, 272] weight, fold all scalar factors
into the weights.

Per-core kernel, per 128-token chunk (8 chunks):
  - load x slab [4096, 128] (1 MiB, contiguous per-partition lines)
  - 1 bias matmul (K=1 ones row x M_b row) + 32 accumulating matmuls
    (stationary = x d-tile, moving = fused W [128, 272]) -> PSUM
    [128 tok, 272] = (M | z)
  - DVE mixing: P = M * broadcast(z), grouped reduce over j -> z_mixed
  - PE transpose z_mixed -> [16, 128], then 8 matmuls against
    B_w.T [16, 4096] -> out rows [128 tok, 4096] in PSUM
  - DVE/ACT evacuate PSUM -> SBUF bf16 (1024-wide ops), DMA store rows
"""

import sys

if "/opt/trn_rl_repo" not in sys.path:
    sys.path.insert(0, "/opt/trn_rl_repo")

import ml_dtypes
import numpy as np

import concourse.bass as bass
import concourse.tile as tile
from concourse import bacc, mybir
from concourse.bass_utils import run_bass_kernel_spmd

N_CORES = 8
B, S, D = 4, 2048, 4096
N_TOK = B * S                # 8192
TPC = N_TOK // N_CORES       # tokens per core = 1024
CHUNK = 128                  # tokens per PSUM chunk
NCHUNK = TPC // CHUNK        # 8
RH = 16                      # lora rank*heads
MDIM = RH * RH               # 256
WCOLS = MDIM + RH            # 272 fused output cols (M | z)
KD = D // 128                # 32 d-chunks
OUT_D = 4096

BF = mybir.dt.bfloat16
F32 = mybir.dt.float32
NPBF = ml_dtypes.bfloat16


def build_nc():
    nc = bacc.Bacc("TRN2", target_bir_lowering=False, debug=False)
    # host-swizzled x: xsw[p, c*(KD*CHUNK) + k*CHUNK + t] = x[c*CHUNK + t, k*128 + p]
    xsw = nc.dram_tensor("xsw", [128, NCHUNK * KD * CHUNK], BF, kind="ExternalInput").ap()
    # host-swizzled W: wsw[p, k*WCOLS + m] = W.T[k*128 + p, m]
    wsw = nc.dram_tensor("wsw", [128, KD * WCOLS], BF, kind="ExternalInput").ap()
    mbr = nc.dram_tensor("mbr", [1, WCOLS], BF, kind="ExternalInput").ap()
    bT = nc.dram_tensor("bT", [RH, OUT_D], BF, kind="ExternalInput").ap()
    ones = nc.dram_tensor("ones", [1, CHUNK], BF, kind="ExternalInput").ap()
    ident = nc.dram_tensor("ident", [CHUNK, CHUNK], F32, kind="ExternalInput").ap()
    out = nc.dram_tensor("out", [TPC, OUT_D], BF, kind="ExternalOutput").ap()

    SLAB = KD * CHUNK  # 4096 cols per token-slab

    with tile.TileContext(nc) as tc:
        with (
            tc.tile_pool(name="xpool", bufs=3) as xpool,
            tc.tile_pool(name="wpool", bufs=1) as wpool,
            tc.tile_pool(name="cpool", bufs=1) as cpool,
            tc.tile_pool(name="mix", bufs=2) as mixpool,
            tc.tile_pool(name="osb", bufs=3) as opool,
            tc.tile_pool(name="am", bufs=3, space="PSUM") as ampool,
            tc.tile_pool(name="tp", bufs=1, space="PSUM") as tpool,
            tc.tile_pool(name="bp", bufs=2, space="PSUM") as bpool,
        ):
            # weights / constants on the SWDGE (gpsimd) queue — concurrent
            # with the x slab loads on the SP HWDGE queue.  Small consts
            # first (the bias matmul opens every accumulation group), then
            # the W quarters.
            mbsb = cpool.tile([1, WCOLS], BF)
            nc.gpsimd.dma_start(mbsb[:], mbr)
            onesb = cpool.tile([1, CHUNK], BF)
            nc.gpsimd.dma_start(onesb[:], ones)
            btsb = cpool.tile([RH, OUT_D], BF)
            nc.gpsimd.dma_start(btsb[:], bT)
            idsb = cpool.tile([CHUNK, CHUNK], F32)
            nc.gpsimd.dma_start(idsb[:], ident)
            wsb = wpool.tile([128, KD, WCOLS], BF)
            wflat = wsb[:].rearrange("p k m -> p (k m)")
            WQ = 8 * WCOLS
            for q in range(4):
                nc.gpsimd.dma_start(wflat[:, q * WQ:(q + 1) * WQ], wsw[:, q * WQ:(q + 1) * WQ])

            for c in range(NCHUNK):
                tok = slice(c * CHUNK, (c + 1) * CHUNK)
                # x slab: all of d for tokens [c*128, (c+1)*128)
                xs = xpool.tile([128, KD, CHUNK], BF)
                nc.sync.dma_start(
                    xs[:].rearrange("p k t -> p (k t)"),
                    xsw[:, c * SLAB:(c + 1) * SLAB],
                )

                # fused A/M matmul: out [128 tok, 272], contract over d.
                # bias row first (K=1, only needs mbr+ones), then 32 d-tiles.
                am = ampool.tile([128, WCOLS], F32)
                nc.tensor.matmul(am[:], lhsT=onesb[:], rhs=mbsb[:], start=True, stop=False)
                for k in range(KD):
                    nc.tensor.matmul(
                        am[:], lhsT=xs[:, k, :], rhs=wsb[:, k, :],
                        start=False, stop=(k == KD - 1),
                    )

                # z -> SBUF (scalar engine, tiny)
                z_sb = mixpool.tile([128, RH], F32, tag="z")
                nc.scalar.copy(z_sb[:], am[:, MDIM:WCOLS])

                # P[p, i, j] = M[p, i, j] * z[p, j]
                p_sb = mixpool.tile([128, MDIM], BF, tag="p")
                nc.vector.tensor_mul(
                    p_sb[:].rearrange("p (i j) -> p i j", i=RH),
                    am[:, 0:MDIM].rearrange("p (i j) -> p i j", i=RH),
                    z_sb[:].unsqueeze(1).broadcast_to([128, RH, RH]),
                )
                # z_mixed[p, i] = sum_j P[p, i, j]
                zm = mixpool.tile([128, RH], F32, tag="zm")
                nc.vector.tensor_reduce(
                    zm[:], p_sb[:].rearrange("p (i j) -> p i j", i=RH),
                    axis=mybir.AxisListType.X, op=mybir.AluOpType.add,
                )

                # transpose z_mixed -> [16, 128] for the B matmul stationary
                zt_ps = tpool.tile([RH, CHUNK], F32)
                nc.tensor.transpose(zt_ps[:], zm[:], idsb[:])
                zt_sb = mixpool.tile([RH, CHUNK], BF, tag="zt")
                nc.scalar.copy(zt_sb[:], zt_ps[:])

                # out rows = z_mixed @ B_w.T : 8 matmuls of [16,128].T @ [16,512]
                o_sb = opool.tile([128, OUT_D], BF)
                for h in range(4):
                    bp = bpool.tile([128, 1024], F32)  # 2 PSUM banks
                    for j in range(2):
                        ob = 2 * h + j
                        nc.tensor.matmul(
                            bp[:, j * 512:(j + 1) * 512],
                            lhsT=zt_sb[:], rhs=btsb[:, ob * 512:(ob + 1) * 512],
                            start=True, stop=True,
                        )
                    osl = slice(h * 1024, (h + 1) * 1024)
                    # split PSUM evacuation between DVE and ACT
                    if h % 2 == 0:
                        nc.vector.tensor_copy(o_sb[:, osl], bp[:])
                    else:
                        nc.scalar.copy(o_sb[:, osl], bp[:])
                # stores on the ACT HWDGE queue — independent FIFO from x loads
                nc.scalar.dma_start(out[tok, :], o_sb[:])

    nc.compile()
    return nc


_NC = None


def _get_nc():
    global _NC
    if _NC is None:
        _NC = build_nc()
    return _NC


def make_in_maps(x, A_w, B_w, M_w, M_b):
    x = np.asarray(x, dtype=np.float32)
    A_w = np.asarray(A_w, dtype=np.float32)
    B_w = np.asarray(B_w, dtype=np.float32)
    M_w = np.asarray(M_w, dtype=np.float32)
    M_b = np.asarray(M_b, dtype=np.float32)

    # fold scales: M' = x @ (8 M_w).T + 8 M_b ; out = z_mixed @ (128 B_w).T
    W = np.concatenate([8.0 * M_w, A_w], axis=0)              # [272, 4096]
    wT_np = W.T.astype(NPBF)                                  # [4096, 272]
    # swizzle to [128, k*272 + m] so each SBUF partition line is contiguous
    wsw_np = np.ascontiguousarray(
        wT_np.reshape(KD, 128, WCOLS).transpose(1, 0, 2).reshape(128, KD * WCOLS)
    )
    mb_np = np.concatenate([8.0 * M_b, np.zeros(RH, np.float32)]).reshape(1, WCOLS).astype(NPBF)
    bT_np = np.ascontiguousarray((128.0 * B_w).T).astype(NPBF)  # [16, 4096]
    ones_np = np.ones((1, CHUNK), dtype=NPBF)
    id_np = np.eye(CHUNK, dtype=np.float32)

    xf = x.reshape(N_TOK, D)
    in_maps = []
    for core in range(N_CORES):
        shard = xf[core * TPC:(core + 1) * TPC].astype(NPBF)  # [1024, 4096]
        # xsw[p, c*4096 + k*128 + t] = shard[c*128 + t, k*128 + p]
        xsw_np = np.ascontiguousarray(
            shard.reshape(NCHUNK, CHUNK, KD, 128)             # [c, t, k, p]
            .transpose(3, 0, 2, 1)                            # [p, c, k, t]
            .reshape(128, NCHUNK * KD * CHUNK)
        )
        in_maps.append({
            "xsw": xsw_np, "wsw": wsw_np, "mbr": mb_np, "bT": bT_np,
            "ones": ones_np, "ident": id_np,
        })
    return in_maps


def assemble_out(results):
    outs = [np.asarray(results[i]["out"], dtype=np.float32) for i in range(N_CORES)]
    return np.concatenate(outs, axis=0).reshape(B, S, OUT_D)


def kernel(x, A_w, B_w, M_w, M_b):
    nc = _get_nc()
    in_maps = make_in_maps(x, A_w, B_w, M_w, M_b)
    res = run_bass_kernel_spmd(nc, in_maps, core_ids=list(range(N_CORES)))
    return assemble_out(res.results)
